# revision 29
# baseline (speedup 1.0000x reference)
"""Causal self-attention (B=2, S=2048, D=1024, H=16, RoPE) on 8 TRN2 NeuronCores.

Sharding: data-parallel over batch (2) x tensor-parallel over head groups (4),
so each core handles one (batch, 4-head group). The QKV projection is
column-sharded, the output projection row-sharded; each core returns a partial
y^T [D, S] in bf16 and the host sums the 4 partials per batch and transposes.

Per-core kernel (bf16 matmuls with fp32 PSUM accumulation), fully interleaved
over token chunks t of 512 so PE/ACT/DVE/DMA pipeline across phases:

  per t: A(t): qkv projection for tokens [512t, 512t+512) (q first so its
           RoPE + head-major repack DMA overlaps the k/v matmuls), RoPE on
           DVE (f32 from PSUM), v to token-major with appended ones column.
         C(t-1): output projection for the previous chunk (emitted here so
           its PE work fills the gap while B(t)'s first scores wait on the
           q repack), y staged to bf16 via DVE and DMA'd out.
         B(j=t): per head: scoresT strips [128 k, 1024 q] via PE, exp via
           ACT (scale=1/8 folded in; scores ~N(0,1) so no row-max needed),
           causal handling at 128-col granularity: diagonal chunks narrow
           the score/att@v matmuls to the unmasked column suffix and only
           the 128x128 tril corner needs a mask multiply (DVE). att@v
           accumulates [65, 512] (row 64 = Z via ones column of v').
           Normalize: reciprocal_approx_fast (DVE custom op, ~5x faster
           than reciprocal) + GPSIMD partition broadcast + DVE multiply.
"""

import os
import sys

import ml_dtypes
import numpy as np

_TRN_REPO = "/root/.axon_site/_ro/trn_rl_repo"
if os.path.isdir(_TRN_REPO) and _TRN_REPO not in sys.path:
    sys.path.insert(0, _TRN_REPO)

from contextlib import ExitStack

import concourse.bass as bass
import concourse.tile as tile
from concourse import bacc, library_config, mybir

F32 = mybir.dt.float32
BF16 = mybir.dt.bfloat16

B = 2
S = 2048
D = 1024
H = 16
HG = 4            # heads per core
DH = 64
TS = 512          # token chunk (matmul moving dim)
NT = S // TS      # 4 token chunks
NKC = D // 128    # 8 d_in chunks
SCALE = DH ** -0.5
ROPE_BASE = 10000.0
N_CORES = 8
EXP = mybir.ActivationFunctionType.Exp


def build_nc():
    nc = bacc.Bacc("TRN2", target_bir_lowering=False, debug=False)

    xT = nc.dram_tensor("xT", [D, S], BF16, kind="ExternalInput").ap()
    wq = nc.dram_tensor("wq", [D, 256], BF16, kind="ExternalInput").ap()
    wk = nc.dram_tensor("wk", [D, 256], BF16, kind="ExternalInput").ap()
    wv = nc.dram_tensor("wv", [D, 256], BF16, kind="ExternalInput").ap()
    wo = nc.dram_tensor("wo", [256, D], BF16, kind="ExternalInput").ap()
    cs = nc.dram_tensor("cs", [128, 2 * S], F32, kind="ExternalInput").ap()
    cmask = nc.dram_tensor("cmask", [128, 128], BF16, kind="ExternalInput").ap()
    yT = nc.dram_tensor("yT", [D, S], BF16, kind="ExternalOutput").ap()

    with tile.TileContext(nc) as tc, ExitStack() as ctx:
        nc.gpsimd.load_library(library_config.attn)

        # ---- persistent SBUF ----
        pw = ctx.enter_context(tc.tile_pool(name="pw", bufs=1))
        pconst = ctx.enter_context(tc.tile_pool(name="pconst", bufs=1))
        pqk = ctx.enter_context(tc.tile_pool(name="pqk", bufs=1))
        pv = ctx.enter_context(tc.tile_pool(name="pv", bufs=1))
        pao = ctx.enter_context(tc.tile_pool(name="pao", bufs=1))

        # weights: [128, 8*256] with d_in chunk k at free offset 256k
        wq_sb = pw.tile([128, NKC * 256], BF16, tag="wq", name="wq")
        wk_sb = pw.tile([128, NKC * 256], BF16, tag="wk", name="wk")
        wv_sb = pw.tile([128, NKC * 256], BF16, tag="wv", name="wv")
        wo_sb = pw.tile([128, 2 * D], BF16, tag="wo", name="wo")
        # one DMA: transfers spread across engines, and a single issue keeps
        # the SP sequencer free for the x tiles + repack DMAs behind it
        # (issue serialization ~600ns each is what delays B(0)'s start)
        nc.sync.dma_start(
            wq_sb[:].rearrange("p (k m) -> p k m", k=NKC),
            wq.rearrange("(k p) m -> p k m", p=128),
        )

        cs_sb = pconst.tile([128, 2 * S], F32, tag="cs", name="cs")

        def load_cs_chunk(t):
            # cos/sin for token chunk t only — keeps the t=0 RoPE off the
            # critical path of one big 2MB table load
            for base in (0, S):
                sl = slice(base + TS * t, base + TS * (t + 1))
                nc.sync.dma_start(cs_sb[:, sl], cs[:, sl])

        def load_rest():
            nc.sync.dma_start(
                wk_sb[:].rearrange("p (k m) -> p k m", k=NKC),
                wk.rearrange("(k p) m -> p k m", p=128),
            )
            # the rest is not needed for ~15us+: issue from the Activation
            # queue (idle during the prologue) so the SP sequencer reaches
            # the q/k repack DMAs — B(0)'s gating dependency — sooner
            nc.scalar.dma_start(
                wv_sb[:].rearrange("p (k m) -> p k m", k=NKC),
                wv.rearrange("(k p) m -> p k m", p=128),
            )
            # causal tril block: cmask[p, x] = 1 if p <= x else 0
            cmask_sb = pconst.tile([128, 128], BF16, tag="cmask", name="cmask")
            nc.scalar.dma_start(cmask_sb[:], cmask)
            for t in range(1, NT):
                for base in (0, S):
                    sl = slice(base + TS * t, base + TS * (t + 1))
                    nc.scalar.dma_start(cs_sb[:, sl], cs[:, sl])
            nc.scalar.dma_start(
                wo_sb[:].rearrange("p (k m) -> p k m", k=2),
                wo.rearrange("(k p) m -> p k m", p=128),
            )
            return cmask_sb

        # q/k head-major: tensor i holds heads 2i (parts 0-63) and 2i+1
        # (64-127); within a head, rotated lo dims at +0, hi dims at +32
        q_hm = [pqk.tile([128, S], BF16, tag=f"q_hm{i}", name=f"q_hm{i}") for i in range(2)]
        k_hm = [pqk.tile([128, S], BF16, tag=f"k_hm{i}", name=f"k_hm{i}") for i in range(2)]

        # v token-major with ones column: chunk tc16 at free offset 260*tc16,
        # head h at +65*h (64 features + 1 ones col)
        v_sb = pv.tile([128, 16 * 260], BF16, tag="v", name="v")
        v_chunks = v_sb[:].rearrange("p (t r) -> p t r", t=16)
        for h in range(HG):
            nc.vector.memset(v_chunks[:, :, 65 * h + 64 : 65 * h + 65], 1.0)

        # attention output, feature-major: head pair hp at free offset S*hp,
        # head h%2 on partitions (h%2)*64..+64
        ao_sb = pao.tile([128, 2 * S], BF16, tag="ao", name="ao")

        with (
            tc.tile_pool(name="px", bufs=16) as px,
            tc.tile_pool(name="ptmp", bufs=2) as ptmp,
            tc.tile_pool(name="pro", bufs=2) as pro,
            tc.tile_pool(name="pp", bufs=4) as pp,
            tc.tile_pool(name="py", bufs=3) as py,
            tc.tile_pool(name="pz", bufs=2) as pz,
            tc.tile_pool(name="psS", bufs=2, space="PSUM") as psS,
            tc.tile_pool(name="psO", bufs=2, space="PSUM") as psO,
            tc.tile_pool(name="psM", bufs=2, space="PSUM") as psM,
        ):
            def emit_phase_c(t):
                for of in range(D // 128):
                    y_ps = psM.tile([128, TS], F32, tag="pj", name="yps")
                    for hp in range(2):
                        nc.tensor.matmul(
                            y_ps[:],
                            wo_sb[:, D * hp + 128 * of : D * hp + 128 * (of + 1)],
                            ao_sb[:, S * hp + TS * t : S * hp + TS * (t + 1)],
                            start=(hp == 0),
                            stop=(hp == 1),
                        )
                    y_sb = py.tile([128, TS], BF16, tag="ysb", name="ysb")
                    nc.vector.tensor_copy(y_sb[:], y_ps[:])
                    nc.sync.dma_start(
                        yT[128 * of : 128 * (of + 1), TS * t : TS * (t + 1)], y_sb[:]
                    )

            cmask_sb = None

            def emit_phase_a(t):
                # ---- A(t): projections + RoPE + repack ----
                nonlocal cmask_sb
                xt = [px.tile([128, TS], BF16, tag="xt", name="xt") for _ in range(NKC)]
                for k in range(NKC):
                    nc.sync.dma_start(
                        xt[k][:], xT[128 * k : 128 * (k + 1), TS * t : TS * (t + 1)]
                    )
                if t == 0:
                    load_cs_chunk(0)
                    # keep-warm: the HAM clock gate drops to 4/8 when the PE
                    # idles; a burst of matmuls on the already-loaded wq
                    # chunk 0 bridges the gap while the x tiles land
                    warm_ps = psM.tile([128, TS], F32, tag="pj", name="warm")
                    for _ in range(16):
                        nc.tensor.matmul(
                            warm_ps[:],
                            wq_sb[:, 0:128],
                            wq_sb[:, 0:TS],
                            start=True,
                            stop=True,
                        )
                    cmask_sb = load_rest()

                cosx = cs_sb[:, TS * t : TS * (t + 1)]
                sinx = cs_sb[:, S + TS * t : S + TS * (t + 1)]

                def emit_qk(w_sb_, hm):
                    ps = [psM.tile([128, TS], F32, tag="pj", name="proj") for _ in range(2)]
                    for fb in range(2):
                        for k in range(NKC):
                            nc.tensor.matmul(
                                ps[fb][:],
                                w_sb_[:, 256 * k + 128 * fb : 256 * k + 128 * (fb + 1)],
                                xt[k][:],
                                start=(k == 0),
                                stop=(k == NKC - 1),
                            )
                    # RoPE: lo' = lo*cos - hi*sin ; hi' = hi*cos + lo*sin
                    t1 = ptmp.tile([128, TS], F32, tag="t1", name="t1")
                    t2 = ptmp.tile([128, TS], F32, tag="t2", name="t2")
                    t3 = ptmp.tile([128, TS], F32, tag="t3", name="t3")
                    t4 = ptmp.tile([128, TS], F32, tag="t4", name="t4")
                    # rotated output: lo halves in cols [0, 512), hi in
                    # [512, 1024) so one tile feeds the batched repack DMA
                    r = pro.tile([128, 2 * TS], BF16, tag="r", name="r")
                    rlo = r[:, 0:TS]
                    rhi = r[:, TS : 2 * TS]
                    # lo half first so its repack DMAs issue ~2 DVE ops
                    # earlier; repack to head-major via SBUF-to-SBUF DMA:
                    # r rows 32h..32h+31 hold head h's rotated dims, lo in
                    # cols [0,512) / hi in [512,1024); hm[h//2] partitions
                    # 64*(h%2)+(0..31) get lo, +(32..63) get hi
                    sl = slice(TS * t, TS * (t + 1))
                    nc.vector.tensor_mul(t1[:], ps[0][:], cosx)
                    nc.vector.tensor_mul(t2[:], ps[1][:], sinx)
                    nc.vector.tensor_sub(rlo, t1[:], t2[:])
                    for h in range(HG):
                        nc.sync.dma_start(
                            hm[h // 2][64 * (h % 2) : 64 * (h % 2) + 32, sl],
                            r[32 * h : 32 * h + 32, 0:TS],
                        )
                    nc.vector.tensor_mul(t3[:], ps[1][:], cosx)
                    nc.vector.tensor_mul(t4[:], ps[0][:], sinx)
                    nc.vector.tensor_add(rhi, t3[:], t4[:])
                    for h in range(HG):
                        bp = 64 * (h % 2) + 32
                        nc.sync.dma_start(
                            hm[h // 2][bp : bp + 32, sl],
                            r[32 * h : 32 * h + 32, TS : 2 * TS],
                        )

                # q first: its RoPE/repack chain overlaps the k/v matmuls
                emit_qk(wq_sb, q_hm)
                emit_qk(wk_sb, k_hm)

                # v projection: token-major, 4 chunks of 128 tokens
                for tc4 in range(4):
                    tc16 = 4 * t + tc4
                    pvps = psM.tile([128, 256], F32, tag="pj", name="vps")
                    for k in range(NKC):
                        nc.tensor.matmul(
                            pvps[:],
                            xt[k][:, 128 * tc4 : 128 * (tc4 + 1)],
                            wv_sb[:, 256 * k : 256 * (k + 1)],
                            start=(k == 0),
                            stop=(k == NKC - 1),
                        )
                    nc.scalar.copy(
                        v_sb[:, 260 * tc16 : 260 * tc16 + 260]
                        .rearrange("p (h m) -> p h m", h=4)[:, :, 0:64],
                        pvps[:].rearrange("p (h m) -> p h m", h=4),
                    )

            def emit_phase_b(j):
                # ---- B(j): attention ----
                for h in range(HG):
                    qh = q_hm[h // 2]
                    kh = k_hm[h // 2]
                    vp = 64 * (h % 2)
                    nch = 4 * j + 4   # causal k-chunks
                    nst = nch // 2    # strips of 2 k-chunks each
                    out_ps = psO.tile([65, TS], F32, tag="ops", name="ops")
                    s_ps = {}
                    p_sb = {}

                    def emit_score(st):
                        # one [128, 1024] PSUM strip = 2 score tiles; diagonal
                        # chunks (m >= 0) narrow to the unmasked column suffix
                        s_ps[st] = psS.tile([128, 2 * TS], F32, tag="sps", name="sps")
                        for u in range(2):
                            c = 2 * st + u
                            q0 = 128 * max(c - 4 * j, 0)
                            nc.tensor.matmul(
                                s_ps[st][:, TS * u + q0 : TS * (u + 1)],
                                kh[vp : vp + 64, 128 * c : 128 * (c + 1)],
                                qh[vp : vp + 64, TS * j + q0 : TS * (j + 1)],
                                start=True,
                                stop=True,
                            )

                    def emit_exp_mask(st):
                        p_sb[st] = pp.tile([128, 2 * TS], BF16, tag="p", name="p")
                        if 2 * st >= 4 * j:
                            # diagonal strip: per-chunk exp over the written
                            # (unmasked-suffix) region only
                            for u in range(2):
                                m = 2 * st + u - 4 * j
                                q0 = TS * u + 128 * m
                                nc.scalar.activation(
                                    p_sb[st][:, q0 : TS * (u + 1)],
                                    s_ps[st][:, q0 : TS * (u + 1)],
                                    EXP,
                                    scale=SCALE,
                                )
                                # tril the 128x128 diagonal corner (DVE: low
                                # latency matters — this sits in the
                                # exp->mask->att@v chain, and PE stalls drop
                                # the HAM clock gate to 4/8)
                                nc.vector.tensor_mul(
                                    p_sb[st][:, q0 : q0 + 128],
                                    p_sb[st][:, q0 : q0 + 128],
                                    cmask_sb[:, 0:128],
                                )
                        else:
                            nc.scalar.activation(
                                p_sb[st][:], s_ps[st][:], EXP, scale=SCALE
                            )
                        s_ps.pop(st)

                    def emit_av(st):
                        for u in range(2):
                            c = 2 * st + u
                            q0 = 128 * max(c - 4 * j, 0)
                            nc.tensor.matmul(
                                out_ps[:, q0:TS],
                                v_sb[:, 260 * c + 65 * h : 260 * c + 65 * (h + 1)],
                                p_sb[st][:, TS * u + q0 : TS * (u + 1)],
                                start=(c == 0),
                                stop=(c == nch - 1),
                            )
                        p_sb.pop(st)

                    LOOKAHEAD = 1
                    for st in range(min(LOOKAHEAD, nst)):
                        emit_score(st)
                    for st in range(nst):
                        emit_exp_mask(st)
                        if st + LOOKAHEAD < nst:
                            emit_score(st + LOOKAHEAD)
                        emit_av(st)

                    # normalize by Z (row 64) into ao_sb; stage Z to SBUF
                    # partition 0 first (the custom-DVE recip mishandles a
                    # PSUM partition-64 source on hardware)
                    zc = pz.tile([1, TS], F32, tag="zc", name="zc")
                    nc.vector.tensor_copy(zc[:], out_ps[64:65, :])
                    zr = pz.tile([1, TS], F32, tag="zr", name="zr")
                    nc.vector.reciprocal_approx_fast(zr[:], zc[:])
                    zb = pz.tile([64, TS], F32, tag="zb", name="zb")
                    nc.gpsimd.partition_broadcast(zb[:], zr[:])
                    ao_slice = ao_sb[
                        vp : vp + 64,
                        S * (h // 2) + TS * j : S * (h // 2) + TS * (j + 1),
                    ]
                    nc.vector.tensor_mul(ao_slice, out_ps[0:64, :], zb[:])

            # ---- pipeline: A runs one chunk ahead so B(t)'s q/k repack
            # chain completes under B(t-1)'s attention work instead of
            # stalling the PE (each stall also drops the HAM clock gate) ----
            emit_phase_a(0)
            for t in range(NT):
                if t + 1 < NT:
                    emit_phase_a(t + 1)
                if 0 < t < NT - 1:
                    emit_phase_c(t - 1)
                emit_phase_b(t)
            # C(2) emitted after B(3): its long-ready matmuls bridge the
            # final normalize chain (~3.4us) with dense PE work so the HAM
            # clock gate stays at 8/8 into the last output projections
            emit_phase_c(NT - 2)

            # keep the clock gate open across the last normalize chain
            warm2_ps = psM.tile([128, TS], F32, tag="pj", name="warm2")
            for _ in range(4):
                nc.tensor.matmul(
                    warm2_ps[:], wq_sb[:, 0:128], wq_sb[:, 0:TS],
                    start=True, stop=True,
                )
            # ---- final output projection chunk ----
            emit_phase_c(NT - 1)

    nc.compile()
    return nc


def prep_core_inputs(x, w_qkv, w_out):
    """Build the 8 per-core input maps from full inputs."""
    x = np.asarray(x, dtype=np.float32)
    w_qkv = np.asarray(w_qkv, dtype=np.float32)
    w_out = np.asarray(w_out, dtype=np.float32)

    wq_all = w_qkv[:, 0 * D : 1 * D].reshape(D, H, DH)
    wk_all = w_qkv[:, 1 * D : 2 * D].reshape(D, H, DH)
    wv_all = w_qkv[:, 2 * D : 3 * D].reshape(D, H, DH)
    wo_all = w_out.reshape(H, DH, D)

    inv = 1.0 / (ROPE_BASE ** (np.arange(0, DH, 2, dtype=np.float32) / DH))
    t = np.arange(S, dtype=np.float32)
    freqs = np.outer(t, inv)  # [S, 32]
    cosT = np.tile(np.cos(freqs).T.astype(np.float32), (4, 1))  # [128, S]
    sinT = np.tile(np.sin(freqs).T.astype(np.float32), (4, 1))
    cs = np.ascontiguousarray(np.concatenate([cosT, sinT], axis=1))
    p_idx = np.arange(128)[:, None]
    x_idx = np.arange(128)[None, :]
    cmask = (p_idx <= x_idx).astype(np.float32)

    in_maps = []
    for core in range(N_CORES):
        b, g = divmod(core, 4)
        hs = slice(4 * g, 4 * g + 4)

        def qk_perm(w_all):
            hgrp = w_all[:, hs, :]  # [D, 4, 64]
            return np.ascontiguousarray(
                np.concatenate(
                    [hgrp[:, :, :32].reshape(D, 128), hgrp[:, :, 32:].reshape(D, 128)],
                    axis=1,
                )
            )

        in_maps.append(
            {
                "xT": np.ascontiguousarray(x[b].T).astype(ml_dtypes.bfloat16),
                "wq": qk_perm(wq_all).astype(ml_dtypes.bfloat16),
                "wk": qk_perm(wk_all).astype(ml_dtypes.bfloat16),
                "wv": np.ascontiguousarray(wv_all[:, hs, :].reshape(D, 256)).astype(ml_dtypes.bfloat16),
                "wo": np.ascontiguousarray(wo_all[hs].reshape(256, D)).astype(ml_dtypes.bfloat16),
                "cs": cs,
                "cmask": cmask.astype(ml_dtypes.bfloat16),
            }
        )
    return in_maps


def gather_output(results):
    """Sum the 4 per-head-group partials per batch and transpose."""
    y = np.empty((B, S, D), dtype=np.float32)
    for b in range(B):
        acc = results[4 * b]["yT"].astype(np.float32)
        for g in range(1, 4):
            acc = acc + results[4 * b + g]["yT"].astype(np.float32)
        y[b] = acc.T
    return y


_NC_CACHE = None


def kernel(x, w_qkv, w_out):
    global _NC_CACHE
    from concourse.bass_utils import run_bass_kernel_spmd

    if _NC_CACHE is None:
        _NC_CACHE = build_nc()
    in_maps = prep_core_inputs(x, w_qkv, w_out)
    res = run_bass_kernel_spmd(_NC_CACHE, in_maps, list(range(N_CORES)))
    return gather_output(res.results)


if __name__ == "__main__":
    build_nc()
    print("built ok")


# revision 30
# speedup vs baseline: 1.0041x; 1.0041x over previous
"""Causal self-attention (B=2, S=2048, D=1024, H=16, RoPE) on 8 TRN2 NeuronCores.

Sharding: data-parallel over batch (2) x tensor-parallel over head groups (4),
so each core handles one (batch, 4-head group). The QKV projection is
column-sharded, the output projection row-sharded; each core returns a partial
y^T [D, S] in bf16 and the host sums the 4 partials per batch and transposes.

Per-core kernel (bf16 matmuls with fp32 PSUM accumulation), fully interleaved
over token chunks t of 512 so PE/ACT/DVE/DMA pipeline across phases:

  per t: A(t): qkv projection for tokens [512t, 512t+512) (q first so its
           RoPE + head-major repack DMA overlaps the k/v matmuls), RoPE on
           DVE (f32 from PSUM), v to token-major with appended ones column.
         C(t-1): output projection for the previous chunk (emitted here so
           its PE work fills the gap while B(t)'s first scores wait on the
           q repack), y staged to bf16 via DVE and DMA'd out.
         B(j=t): per head: scoresT strips [128 k, 1024 q] via PE, exp via
           ACT (scale=1/8 folded in; scores ~N(0,1) so no row-max needed),
           causal handling at 128-col granularity: diagonal chunks narrow
           the score/att@v matmuls to the unmasked column suffix and only
           the 128x128 tril corner needs a mask multiply (DVE). att@v
           accumulates [65, 512] (row 64 = Z via ones column of v').
           Normalize: reciprocal_approx_fast (DVE custom op, ~5x faster
           than reciprocal) + GPSIMD partition broadcast + DVE multiply.
"""

import os
import sys

import ml_dtypes
import numpy as np

_TRN_REPO = "/root/.axon_site/_ro/trn_rl_repo"
if os.path.isdir(_TRN_REPO) and _TRN_REPO not in sys.path:
    sys.path.insert(0, _TRN_REPO)

from contextlib import ExitStack

import concourse.bass as bass
import concourse.tile as tile
from concourse import bacc, library_config, mybir

F32 = mybir.dt.float32
BF16 = mybir.dt.bfloat16

B = 2
S = 2048
D = 1024
H = 16
HG = 4            # heads per core
DH = 64
TS = 512          # token chunk (matmul moving dim)
NT = S // TS      # 4 token chunks
NKC = D // 128    # 8 d_in chunks
SCALE = DH ** -0.5
ROPE_BASE = 10000.0
N_CORES = 8
EXP = mybir.ActivationFunctionType.Exp


def build_nc():
    nc = bacc.Bacc("TRN2", target_bir_lowering=False, debug=False)

    xT = nc.dram_tensor("xT", [D, S], BF16, kind="ExternalInput").ap()
    wq = nc.dram_tensor("wq", [D, 256], BF16, kind="ExternalInput").ap()
    wk = nc.dram_tensor("wk", [D, 256], BF16, kind="ExternalInput").ap()
    wv = nc.dram_tensor("wv", [D, 256], BF16, kind="ExternalInput").ap()
    wo = nc.dram_tensor("wo", [256, D], BF16, kind="ExternalInput").ap()
    cs = nc.dram_tensor("cs", [128, 2 * S], F32, kind="ExternalInput").ap()
    cmask = nc.dram_tensor("cmask", [128, 128], BF16, kind="ExternalInput").ap()
    yT = nc.dram_tensor("yT", [D, S], BF16, kind="ExternalOutput").ap()

    with tile.TileContext(nc) as tc, ExitStack() as ctx:
        nc.gpsimd.load_library(library_config.attn)

        # ---- persistent SBUF ----
        pw = ctx.enter_context(tc.tile_pool(name="pw", bufs=1))
        pconst = ctx.enter_context(tc.tile_pool(name="pconst", bufs=1))
        pqk = ctx.enter_context(tc.tile_pool(name="pqk", bufs=1))
        pv = ctx.enter_context(tc.tile_pool(name="pv", bufs=1))
        pao = ctx.enter_context(tc.tile_pool(name="pao", bufs=1))

        # weights: [128, 8*256] with d_in chunk k at free offset 256k
        wq_sb = pw.tile([128, NKC * 256], BF16, tag="wq", name="wq")
        wk_sb = pw.tile([128, NKC * 256], BF16, tag="wk", name="wk")
        wv_sb = pw.tile([128, NKC * 256], BF16, tag="wv", name="wv")
        wo_sb = pw.tile([128, 2 * D], BF16, tag="wo", name="wo")
        # one DMA: transfers spread across engines, and a single issue keeps
        # the SP sequencer free for the x tiles + repack DMAs behind it
        # (issue serialization ~600ns each is what delays B(0)'s start)
        nc.sync.dma_start(
            wq_sb[:].rearrange("p (k m) -> p k m", k=NKC),
            wq.rearrange("(k p) m -> p k m", p=128),
        )

        cs_sb = pconst.tile([128, 2 * S], F32, tag="cs", name="cs")

        def load_cs_chunk(t):
            # cos/sin for token chunk t only — keeps the t=0 RoPE off the
            # critical path of one big 2MB table load
            for base in (0, S):
                sl = slice(base + TS * t, base + TS * (t + 1))
                nc.sync.dma_start(cs_sb[:, sl], cs[:, sl])

        def load_rest():
            nc.sync.dma_start(
                wk_sb[:].rearrange("p (k m) -> p k m", k=NKC),
                wk.rearrange("(k p) m -> p k m", p=128),
            )
            # the rest is not needed for ~15us+: issue from the Activation
            # queue (idle during the prologue) so the SP sequencer reaches
            # the q/k repack DMAs — B(0)'s gating dependency — sooner
            nc.scalar.dma_start(
                wv_sb[:].rearrange("p (k m) -> p k m", k=NKC),
                wv.rearrange("(k p) m -> p k m", p=128),
            )
            # causal tril block: cmask[p, x] = 1 if p <= x else 0
            cmask_sb = pconst.tile([128, 128], BF16, tag="cmask", name="cmask")
            nc.scalar.dma_start(cmask_sb[:], cmask)
            for t in range(1, NT):
                for base in (0, S):
                    sl = slice(base + TS * t, base + TS * (t + 1))
                    nc.scalar.dma_start(cs_sb[:, sl], cs[:, sl])
            nc.scalar.dma_start(
                wo_sb[:].rearrange("p (k m) -> p k m", k=2),
                wo.rearrange("(k p) m -> p k m", p=128),
            )
            return cmask_sb

        # q/k head-major: tensor i holds heads 2i (parts 0-63) and 2i+1
        # (64-127); within a head, rotated lo dims at +0, hi dims at +32
        q_hm = [pqk.tile([128, S], BF16, tag=f"q_hm{i}", name=f"q_hm{i}") for i in range(2)]
        k_hm = [pqk.tile([128, S], BF16, tag=f"k_hm{i}", name=f"k_hm{i}") for i in range(2)]

        # v token-major with ones column: chunk tc16 at free offset 260*tc16,
        # head h at +65*h (64 features + 1 ones col)
        v_sb = pv.tile([128, 16 * 260], BF16, tag="v", name="v")
        v_chunks = v_sb[:].rearrange("p (t r) -> p t r", t=16)
        for h in range(HG):
            nc.vector.memset(v_chunks[:, :, 65 * h + 64 : 65 * h + 65], 1.0)

        # attention output, feature-major: head pair hp at free offset S*hp,
        # head h%2 on partitions (h%2)*64..+64
        ao_sb = pao.tile([128, 2 * S], BF16, tag="ao", name="ao")

        with (
            tc.tile_pool(name="px", bufs=16) as px,
            tc.tile_pool(name="ptmp", bufs=2) as ptmp,
            tc.tile_pool(name="pro", bufs=2) as pro,
            tc.tile_pool(name="pp", bufs=4) as pp,
            tc.tile_pool(name="py", bufs=3) as py,
            tc.tile_pool(name="pz", bufs=2) as pz,
            tc.tile_pool(name="psS", bufs=2, space="PSUM") as psS,
            tc.tile_pool(name="psO", bufs=2, space="PSUM") as psO,
            tc.tile_pool(name="psM", bufs=2, space="PSUM") as psM,
        ):
            def emit_phase_c(t):
                for of in range(D // 128):
                    y_ps = psM.tile([128, TS], F32, tag="pj", name="yps")
                    for hp in range(2):
                        nc.tensor.matmul(
                            y_ps[:],
                            wo_sb[:, D * hp + 128 * of : D * hp + 128 * (of + 1)],
                            ao_sb[:, S * hp + TS * t : S * hp + TS * (t + 1)],
                            start=(hp == 0),
                            stop=(hp == 1),
                        )
                    y_sb = py.tile([128, TS], BF16, tag="ysb", name="ysb")
                    nc.vector.tensor_copy(y_sb[:], y_ps[:])
                    nc.sync.dma_start(
                        yT[128 * of : 128 * (of + 1), TS * t : TS * (t + 1)], y_sb[:]
                    )

            cmask_sb = None

            def emit_phase_a(t):
                # ---- A(t): projections + RoPE + repack ----
                nonlocal cmask_sb
                xt = [px.tile([128, TS], BF16, tag="xt", name="xt") for _ in range(NKC)]
                for k in range(NKC):
                    nc.sync.dma_start(
                        xt[k][:], xT[128 * k : 128 * (k + 1), TS * t : TS * (t + 1)]
                    )
                if t == 0:
                    load_cs_chunk(0)
                    # keep-warm: the HAM clock gate drops to 4/8 when the PE
                    # idles; a burst of matmuls on the already-loaded wq
                    # chunk 0 bridges the gap while the x tiles land
                    warm_ps = psM.tile([128, TS], F32, tag="pj", name="warm")
                    for _ in range(16):
                        nc.tensor.matmul(
                            warm_ps[:],
                            wq_sb[:, 0:128],
                            wq_sb[:, 0:TS],
                            start=True,
                            stop=True,
                        )
                    cmask_sb = load_rest()

                cosx = cs_sb[:, TS * t : TS * (t + 1)]
                sinx = cs_sb[:, S + TS * t : S + TS * (t + 1)]

                def emit_qk(w_sb_, hm):
                    ps = [psM.tile([128, TS], F32, tag="pj", name="proj") for _ in range(2)]
                    for fb in range(2):
                        for k in range(NKC):
                            nc.tensor.matmul(
                                ps[fb][:],
                                w_sb_[:, 256 * k + 128 * fb : 256 * k + 128 * (fb + 1)],
                                xt[k][:],
                                start=(k == 0),
                                stop=(k == NKC - 1),
                            )
                    # RoPE: lo' = lo*cos - hi*sin ; hi' = hi*cos + lo*sin
                    t1 = ptmp.tile([128, TS], F32, tag="t1", name="t1")
                    t2 = ptmp.tile([128, TS], F32, tag="t2", name="t2")
                    t3 = ptmp.tile([128, TS], F32, tag="t3", name="t3")
                    t4 = ptmp.tile([128, TS], F32, tag="t4", name="t4")
                    # rotated output: lo halves in cols [0, 512), hi in
                    # [512, 1024) so one tile feeds the batched repack DMA
                    r = pro.tile([128, 2 * TS], BF16, tag="r", name="r")
                    rlo = r[:, 0:TS]
                    rhi = r[:, TS : 2 * TS]
                    # lo half first so its repack DMAs issue ~2 DVE ops
                    # earlier; repack to head-major via SBUF-to-SBUF DMA:
                    # r rows 32h..32h+31 hold head h's rotated dims, lo in
                    # cols [0,512) / hi in [512,1024); hm[h//2] partitions
                    # 64*(h%2)+(0..31) get lo, +(32..63) get hi
                    sl = slice(TS * t, TS * (t + 1))
                    nc.vector.tensor_mul(t1[:], ps[0][:], cosx)
                    nc.vector.tensor_mul(t2[:], ps[1][:], sinx)
                    nc.vector.tensor_sub(rlo, t1[:], t2[:])
                    for h in range(HG):
                        nc.sync.dma_start(
                            hm[h // 2][64 * (h % 2) : 64 * (h % 2) + 32, sl],
                            r[32 * h : 32 * h + 32, 0:TS],
                        )
                    nc.vector.tensor_mul(t3[:], ps[1][:], cosx)
                    nc.vector.tensor_mul(t4[:], ps[0][:], sinx)
                    nc.vector.tensor_add(rhi, t3[:], t4[:])
                    for h in range(HG):
                        bp = 64 * (h % 2) + 32
                        nc.sync.dma_start(
                            hm[h // 2][bp : bp + 32, sl],
                            r[32 * h : 32 * h + 32, TS : 2 * TS],
                        )

                # q first: its RoPE/repack chain overlaps the k/v matmuls
                emit_qk(wq_sb, q_hm)
                emit_qk(wk_sb, k_hm)

                # v projection: token-major, 4 chunks of 128 tokens
                for tc4 in range(4):
                    tc16 = 4 * t + tc4
                    pvps = psM.tile([128, 256], F32, tag="pj", name="vps")
                    for k in range(NKC):
                        nc.tensor.matmul(
                            pvps[:],
                            xt[k][:, 128 * tc4 : 128 * (tc4 + 1)],
                            wv_sb[:, 256 * k : 256 * (k + 1)],
                            start=(k == 0),
                            stop=(k == NKC - 1),
                        )
                    nc.scalar.copy(
                        v_sb[:, 260 * tc16 : 260 * tc16 + 260]
                        .rearrange("p (h m) -> p h m", h=4)[:, :, 0:64],
                        pvps[:].rearrange("p (h m) -> p h m", h=4),
                    )

            def emit_phase_b(j):
                # ---- B(j): attention ----
                for h in range(HG):
                    qh = q_hm[h // 2]
                    kh = k_hm[h // 2]
                    vp = 64 * (h % 2)
                    nch = 4 * j + 4   # causal k-chunks
                    nst = nch // 2    # strips of 2 k-chunks each
                    out_ps = psO.tile([65, TS], F32, tag="ops", name="ops")
                    s_ps = {}
                    p_sb = {}

                    def emit_score(st):
                        # one [128, 1024] PSUM strip = 2 score tiles; diagonal
                        # chunks (m >= 0) narrow to the unmasked column suffix
                        s_ps[st] = psS.tile([128, 2 * TS], F32, tag="sps", name="sps")
                        for u in range(2):
                            c = 2 * st + u
                            q0 = 128 * max(c - 4 * j, 0)
                            nc.tensor.matmul(
                                s_ps[st][:, TS * u + q0 : TS * (u + 1)],
                                kh[vp : vp + 64, 128 * c : 128 * (c + 1)],
                                qh[vp : vp + 64, TS * j + q0 : TS * (j + 1)],
                                start=True,
                                stop=True,
                            )

                    def emit_exp_mask(st):
                        p_sb[st] = pp.tile([128, 2 * TS], BF16, tag="p", name="p")
                        if 2 * st >= 4 * j:
                            # diagonal strip: per-chunk exp over the written
                            # (unmasked-suffix) region only
                            for u in range(2):
                                m = 2 * st + u - 4 * j
                                q0 = TS * u + 128 * m
                                nc.scalar.activation(
                                    p_sb[st][:, q0 : TS * (u + 1)],
                                    s_ps[st][:, q0 : TS * (u + 1)],
                                    EXP,
                                    scale=SCALE,
                                )
                                # tril the 128x128 diagonal corner (DVE: low
                                # latency matters — this sits in the
                                # exp->mask->att@v chain, and PE stalls drop
                                # the HAM clock gate to 4/8)
                                nc.vector.tensor_mul(
                                    p_sb[st][:, q0 : q0 + 128],
                                    p_sb[st][:, q0 : q0 + 128],
                                    cmask_sb[:, 0:128],
                                )
                        else:
                            nc.scalar.activation(
                                p_sb[st][:], s_ps[st][:], EXP, scale=SCALE
                            )
                        s_ps.pop(st)

                    def emit_av(st):
                        for u in range(2):
                            c = 2 * st + u
                            q0 = 128 * max(c - 4 * j, 0)
                            nc.tensor.matmul(
                                out_ps[:, q0:TS],
                                v_sb[:, 260 * c + 65 * h : 260 * c + 65 * (h + 1)],
                                p_sb[st][:, TS * u + q0 : TS * (u + 1)],
                                start=(c == 0),
                                stop=(c == nch - 1),
                            )
                        p_sb.pop(st)

                    LOOKAHEAD = 1
                    for st in range(min(LOOKAHEAD, nst)):
                        emit_score(st)
                    for st in range(nst):
                        emit_exp_mask(st)
                        if st + LOOKAHEAD < nst:
                            emit_score(st + LOOKAHEAD)
                        emit_av(st)

                    # normalize by Z (row 64) into ao_sb; stage Z to SBUF
                    # partition 0 first (the custom-DVE recip mishandles a
                    # PSUM partition-64 source on hardware)
                    zc = pz.tile([1, TS], F32, tag="zc", name="zc")
                    nc.vector.tensor_copy(zc[:], out_ps[64:65, :])
                    zr = pz.tile([1, TS], F32, tag="zr", name="zr")
                    nc.vector.reciprocal_approx_fast(zr[:], zc[:])
                    zb = pz.tile([64, TS], F32, tag="zb", name="zb")
                    nc.gpsimd.partition_broadcast(zb[:], zr[:])
                    ao_slice = ao_sb[
                        vp : vp + 64,
                        S * (h // 2) + TS * j : S * (h // 2) + TS * (j + 1),
                    ]
                    nc.vector.tensor_mul(ao_slice, out_ps[0:64, :], zb[:])

            # ---- pipeline: A runs one chunk ahead so B(t)'s q/k repack
            # chain completes under B(t-1)'s attention work instead of
            # stalling the PE (each stall also drops the HAM clock gate) ----
            emit_phase_a(0)
            for t in range(NT):
                if t + 1 < NT:
                    emit_phase_a(t + 1)
                if t > 0:
                    emit_phase_c(t - 1)
                emit_phase_b(t)

            # keep the clock gate open across the last normalize chain
            warm2_ps = psM.tile([128, TS], F32, tag="pj", name="warm2")
            for _ in range(4):
                nc.tensor.matmul(
                    warm2_ps[:], wq_sb[:, 0:128], wq_sb[:, 0:TS],
                    start=True, stop=True,
                )
            # ---- final output projection chunk ----
            emit_phase_c(NT - 1)

    nc.compile()
    return nc


def prep_core_inputs(x, w_qkv, w_out):
    """Build the 8 per-core input maps from full inputs."""
    x = np.asarray(x, dtype=np.float32)
    w_qkv = np.asarray(w_qkv, dtype=np.float32)
    w_out = np.asarray(w_out, dtype=np.float32)

    wq_all = w_qkv[:, 0 * D : 1 * D].reshape(D, H, DH)
    wk_all = w_qkv[:, 1 * D : 2 * D].reshape(D, H, DH)
    wv_all = w_qkv[:, 2 * D : 3 * D].reshape(D, H, DH)
    wo_all = w_out.reshape(H, DH, D)

    inv = 1.0 / (ROPE_BASE ** (np.arange(0, DH, 2, dtype=np.float32) / DH))
    t = np.arange(S, dtype=np.float32)
    freqs = np.outer(t, inv)  # [S, 32]
    cosT = np.tile(np.cos(freqs).T.astype(np.float32), (4, 1))  # [128, S]
    sinT = np.tile(np.sin(freqs).T.astype(np.float32), (4, 1))
    cs = np.ascontiguousarray(np.concatenate([cosT, sinT], axis=1))
    p_idx = np.arange(128)[:, None]
    x_idx = np.arange(128)[None, :]
    cmask = (p_idx <= x_idx).astype(np.float32)

    in_maps = []
    for core in range(N_CORES):
        b, g = divmod(core, 4)
        hs = slice(4 * g, 4 * g + 4)

        def qk_perm(w_all):
            hgrp = w_all[:, hs, :]  # [D, 4, 64]
            return np.ascontiguousarray(
                np.concatenate(
                    [hgrp[:, :, :32].reshape(D, 128), hgrp[:, :, 32:].reshape(D, 128)],
                    axis=1,
                )
            )

        in_maps.append(
            {
                "xT": np.ascontiguousarray(x[b].T).astype(ml_dtypes.bfloat16),
                "wq": qk_perm(wq_all).astype(ml_dtypes.bfloat16),
                "wk": qk_perm(wk_all).astype(ml_dtypes.bfloat16),
                "wv": np.ascontiguousarray(wv_all[:, hs, :].reshape(D, 256)).astype(ml_dtypes.bfloat16),
                "wo": np.ascontiguousarray(wo_all[hs].reshape(256, D)).astype(ml_dtypes.bfloat16),
                "cs": cs,
                "cmask": cmask.astype(ml_dtypes.bfloat16),
            }
        )
    return in_maps


def gather_output(results):
    """Sum the 4 per-head-group partials per batch and transpose."""
    y = np.empty((B, S, D), dtype=np.float32)
    for b in range(B):
        acc = results[4 * b]["yT"].astype(np.float32)
        for g in range(1, 4):
            acc = acc + results[4 * b + g]["yT"].astype(np.float32)
        y[b] = acc.T
    return y


_NC_CACHE = None


def kernel(x, w_qkv, w_out):
    global _NC_CACHE
    from concourse.bass_utils import run_bass_kernel_spmd

    if _NC_CACHE is None:
        _NC_CACHE = build_nc()
    in_maps = prep_core_inputs(x, w_qkv, w_out)
    res = run_bass_kernel_spmd(_NC_CACHE, in_maps, list(range(N_CORES)))
    return gather_output(res.results)


if __name__ == "__main__":
    build_nc()
    print("built ok")
